# revision 52
# baseline (speedup 1.0000x reference)
"""Attention kernel for trn2: B=4, N=2048, DIM=512, HEADS=8, DIM_HEAD=64.

Sharding: head-parallel across 8 cores (core h computes head h for all 4
batches). Each core returns a partial [4, 2048, 512] output (its head's
contribution through W_out); the host sums the 8 partials.

Per-core pipeline (all matmuls bf16 in / fp32 PSUM out):
  phase 1 per batch: transposed-output projection qk^T = [Wq|Wk]^T x
    straight into [dh, n] layout; a second projection with the rotation
    matrix folded into the weights gives the rotate-half term, so rotary
    is 3 wide DVE ops (cos/sin multiply + add) and no PE transposes.
  phase 2 per batch: v projected in natural [n, dh] layout (ones column
    appended for softmax denominators), hidden under the attention
    pipeline; then per 512-wide q tile: S^T[k,q] = kT.T @ qT (K=64, one
    matmul per 128-k chunk, two chunks share a 2-bank PSUM tile); exp on
    ACT over the 2-bank span; multiply by precomputed exp(pos_bias^T)
    (bf16, loaded once, reused all batches) on DVE; PV matmul with
    lhsT=[v|1] accumulates head_out^T and row sums; the sums row is
    transposed via K=1 matmuls; W_out projection per stride-4 q chunk;
    1/sum applied as a per-partition scalar on the final PSUM->SBUF
    copy; DMA out. Epilogues are deferred one q tile so the in-order
    engine queues never stall the exp pipeline.
"""

import numpy as np

B, N, DIM = 4, 2048, 512
HEADS, DH = 8, 64
P = 128
DC = DIM // P          # 4 dim chunks of the contraction
QT = 512               # q tile width in phase 2
NQT = N // QT          # 4
KC = N // P            # 16 k chunks
NT = 512               # token tile width in phase 1
NNT = N // NT          # 4

_CACHE = {}


def _build():
    import concourse.mybir as mybir
    import concourse.tile as tile
    from concourse import bacc

    F32 = mybir.dt.float32
    BF16 = mybir.dt.bfloat16

    nc = bacc.Bacc(None, target_bir_lowering=False)

    xT4_d = nc.dram_tensor("xT4", [B, P, DC, N], BF16, kind="ExternalInput")
    # weights+rotary tables packed into one tensor: [wqk 512 | wqkr 512 |
    # wv 256 | wout 512 | cos2 2048 | sin2 2048] elems per partition
    CW = 2 * DC * P + DC * DH + DIM
    consts_d = nc.dram_tensor("consts", [P, CW + 2 * N], BF16, kind="ExternalInput")
    expb_d = nc.dram_tensor("expb", [P, KC, N], BF16, kind="ExternalInput")
    out_d = nc.dram_tensor("out", [B, N, DIM], F32, kind="ExternalOutput")

    EXP = mybir.ActivationFunctionType.Exp

    with tile.TileContext(nc) as tc:
        with tc.tile_pool(name="const", bufs=1) as cp:
            cst = cp.tile([P, CW + 2 * N], BF16, tag="cst")
            nc.sync.dma_start(cst[:, 0:CW], consts_d[:, 0:CW])
            o = 0
            wqk_t = cst[:, o : o + DC * P].rearrange("p (a c) -> p a c", c=P)
            o += DC * P
            wqkr_t = cst[:, o : o + DC * P].rearrange("p (a c) -> p a c", c=P)
            o += DC * P
            wv_t = cst[:, o : o + DC * DH].rearrange("p (a c) -> p a c", c=DH)
            o += DC * DH
            wout_t = cst[0:DH, o : o + DIM]
            o += DIM
            cos2_t = cst[:, o : o + N]
            o += N
            sin2_t = cst[:, o : o + N]

            expb_t = cp.tile([P, KC, N], BF16, tag="expb")

            qkrot_b = [
                cp.tile([P, N], BF16, tag=f"qkrot{b}", name=f"qkrot{b}")
                for b in range(B)
            ]
            kT_b = [
                cp.tile([DH, N], BF16, tag=f"kT{b}", name=f"kT{b}")
                for b in range(B)
            ]
            v_b = [
                cp.tile([P, KC, DH + 1], BF16, tag=f"v{b}", name=f"v{b}")
                for b in range(B)
            ]
            for b in range(B):
                nc.vector.memset(v_b[b][:, :, DH : DH + 1], 1.0)
            ones_col = cp.tile([P, 1], BF16, tag="ones_col")
            nc.vector.memset(ones_col[:], 1.0)

            with (
                tc.tile_pool(name="p1", bufs=3) as p1,
                tc.tile_pool(name="px", bufs=4) as px,
                tc.tile_pool(name="p2", bufs=4) as p2,
                tc.tile_pool(name="pob", bufs=3) as pob,
                tc.tile_pool(name="tiny", bufs=4) as tiny,
            ):
                # ---- phase 1: projections + rotary ----
                with (
                    tc.tile_pool(name="ps_p1", bufs=4, space="PSUM") as ps_p1,
                ):
                    # all x loads prefetched upfront: none has waits, so
                    # the SP queue never blocks behind a dependency
                    xb_t = {}
                    for b in range(B):
                        xb_t[b] = px.tile([P, DC, N], BF16, tag="xb", name=f"xb{b}")
                    for b in range(B):
                        for nt in range(NNT):
                            nts = slice(nt * NT, (nt + 1) * NT)
                            nc.sync.dma_start(
                                xb_t[b][:, :, nts], xT4_d[b, :, :, nts]
                            )
                            if b == 0 and nt == 0:
                                # rotary tables after the first x tile (they
                                # are first needed a few us in)
                                nc.sync.dma_start(
                                    cst[:, CW:], consts_d[:, CW:]
                                )
                    for b in range(B):
                        xb = xb_t[b]
                        for nt in range(NNT):
                            nts = slice(nt * NT, (nt + 1) * NT)

                            qkrt_ps = ps_p1.tile([P, 2, NT], F32, tag="qkrt")
                            for dc in range(DC):
                                nc.tensor.matmul(
                                    qkrt_ps[:, 0],
                                    lhsT=wqk_t[:, dc],
                                    rhs=xb[:, dc, nts],
                                    start=(dc == 0),
                                    stop=(dc == DC - 1),
                                )
                            # rotate-half term from a second projection with
                            # the rotation matrix folded into the weights
                            for dc in range(DC):
                                nc.tensor.matmul(
                                    qkrt_ps[:, 1],
                                    lhsT=wqkr_t[:, dc],
                                    rhs=xb[:, dc, nts],
                                    start=(dc == 0),
                                    stop=(dc == DC - 1),
                                )
                            m1 = p1.tile([P, NT], BF16, tag="m1")
                            nc.vector.tensor_mul(
                                m1[:], qkrt_ps[:, 0], cos2_t[:, nts]
                            )
                            m2 = p1.tile([P, NT], BF16, tag="m2")
                            nc.vector.tensor_mul(
                                m2[:], qkrt_ps[:, 1], sin2_t[:, nts]
                            )
                            nc.vector.tensor_add(
                                qkrot_b[b][:, nts], m1[:], m2[:]
                            )
                        # k half (partitions 64..127) relocated to partitions
                        # 0..63 so S matmuls get lhsT/rhs on the same base.
                        # On the ACT hwdge queue: on SP it would wait at the
                        # queue head and block the next batch's x loads.
                        nc.scalar.dma_start(kT_b[b][:, :], qkrot_b[b][DH:P, :])

                # ---- phase 2: attention ----
                with (
                    tc.tile_pool(name="ps_s", bufs=2, space="PSUM") as ps_s,
                    tc.tile_pool(name="ps_o", bufs=2, space="PSUM") as ps_o,
                    tc.tile_pool(name="ps_w", bufs=2, space="PSUM") as ps_w,
                ):
                    def epilogue(b, jq, outT_ps):
                        ho = p2.tile([DH + 1, QT], BF16, tag="ho")
                        with tc.high_priority(offset=60):
                            nc.vector.tensor_copy(ho[:], outT_ps[:])
                        # transpose the sums row via K=1 matmuls (column sq
                        # holds sums for q = p*4 + sq): cheaper than a DMA
                        sums_ps = ps_w.tile([P, QT // P], F32, tag="wo")
                        for sq in range(QT // P):
                            nc.tensor.matmul(
                                sums_ps[:, sq : sq + 1],
                                lhsT=ho[DH : DH + 1, sq : QT : QT // P],
                                rhs=ones_col[DH : DH + 1, :],
                                start=True,
                                stop=True,
                            )
                        recipT = tiny.tile([P, QT // P], F32, tag="recipT")
                        with nc.allow_low_precision(reason="softmax denom"):
                            nc.vector.reciprocal(recipT[:], sums_ps[:])
                        for sq in range(QT // P):
                            # q positions strided by 4: chunk sq covers
                            # q = jq*QT + p*4 + sq, matching sumsT layout
                            wo_ps = ps_w.tile([P, DIM], F32, tag="wo")
                            nc.tensor.matmul(
                                wo_ps[:],
                                lhsT=ho[0:DH, sq : QT : QT // P],
                                rhs=wout_t[:],
                                start=True,
                                stop=True,
                            )
                            ob = pob.tile([P, DIM], F32, tag="ob")
                            nc.vector.tensor_scalar_mul(
                                ob[:], wo_ps[:], recipT[:, sq : sq + 1]
                            )
                            row0 = jq * QT + sq
                            nstep = QT // P
                            nc.sync.dma_start(
                                out_d[
                                    b,
                                    row0 : row0 + (P - 1) * nstep + 1 : nstep,
                                    :,
                                ],
                                ob[:],
                            )

                    pending = None
                    for b in range(B):
                        if b == 0:
                            for kc_ld in range(KC):
                                nc.sync.dma_start(
                                    expb_t[:, kc_ld], expb_d[:, kc_ld, :]
                                )
                        # v projection for this batch (reads the resident x
                        # tile); hides under the attention pipeline
                        for nt in range(NNT):
                            v_ps = ps_w.tile([P, NT // P, DH], F32, tag="wo")
                            for tt in range(NT // P):
                                t0 = nt * NT + tt * P
                                for dc in range(DC):
                                    nc.tensor.matmul(
                                        v_ps[:, tt],
                                        lhsT=xb_t[b][:, dc, t0 : t0 + P],
                                        rhs=wv_t[:, dc],
                                        start=(dc == 0),
                                        stop=(dc == DC - 1),
                                    )
                            k0 = nt * (NT // P)
                            nc.vector.tensor_copy(
                                v_b[b][:, k0 : k0 + NT // P, 0:DH], v_ps[:]
                            )
                        for jq in range(NQT):
                            qs = slice(jq * QT, (jq + 1) * QT)
                            outT_ps = ps_o.tile([DH + 1, QT], F32, tag="outT")
                            for kp in range(KC // 2):
                                s_ps = ps_s.tile([P, 2, QT], F32, tag="s")
                                with tc.high_priority(offset=40):
                                    for i in range(2):
                                        kc = kp * 2 + i
                                        nc.tensor.matmul(
                                            s_ps[:, i],
                                            lhsT=kT_b[b][
                                                :, kc * P : (kc + 1) * P
                                            ],
                                            rhs=qkrot_b[b][0:DH, qs],
                                            start=True,
                                            stop=True,
                                        )
                                et = p2.tile([P, 2, QT], BF16, tag="et")
                                nc.scalar.activation(et[:], s_ps[:], EXP)
                                nc.vector.tensor_mul(
                                    et[:],
                                    et[:],
                                    expb_t[:, kp * 2 : kp * 2 + 2, qs],
                                )
                                for i in range(2):
                                    kc = kp * 2 + i
                                    nc.tensor.matmul(
                                        outT_ps[:],
                                        lhsT=v_b[b][:, kc],
                                        rhs=et[:, i],
                                        start=(kc == 0),
                                        stop=(kc == KC - 1),
                                    )
                                # previous q-tile's epilogue, issued mid-loop
                                # so PE's in-order queue doesn't stall
                                if kp == 1 and pending is not None:
                                    pending()
                                    pending = None
                            pending = (
                                lambda b=b, jq=jq, t=outT_ps: epilogue(
                                    b, jq, t
                                )
                            )
                    pending()

    nc.compile()
    return nc


def _host_inputs(x, pos_bias, W_qkv, W_out):
    """Build the per-core input maps (pure data marshalling)."""
    import ml_dtypes

    bf16 = ml_dtypes.bfloat16

    xT = x.transpose(0, 2, 1)                                 # [B, DIM, N]
    xT4 = np.ascontiguousarray(
        xT.reshape(B, DC, P, N).transpose(0, 2, 1, 3)
    ).astype(bf16)                                            # [B, P, DC, N]

    inv_freq = 1.0 / (10000.0 ** (np.arange(0, DH, 2, dtype=np.float32) / DH))
    freqs = np.arange(N, dtype=np.float32)[:, None] * inv_freq[None, :]  # [N, 32]
    freqs = np.repeat(freqs, 2, axis=-1)                      # [N, 64]
    cosT = np.cos(freqs).T                                    # [64, N]
    sinT = np.sin(freqs).T
    cos2 = np.ascontiguousarray(np.vstack([cosT, cosT])).astype(bf16)  # [128, N]
    sin2 = np.ascontiguousarray(np.vstack([sinT, sinT])).astype(bf16)

    def fold_rot(W):
        # W @ R.T: col 2i -> -W[:, 2i+1]; col 2i+1 -> W[:, 2i]
        Wr = np.empty_like(W)
        Wr[:, 0::2] = -W[:, 1::2]
        Wr[:, 1::2] = W[:, 0::2]
        return Wr

    def chunked(Wfull):
        # [512, C] -> [P, DC, C]
        return np.ascontiguousarray(
            Wfull.reshape(DC, P, -1).transpose(1, 0, 2)
        ).astype(bf16)

    scale = np.float32(DH**-0.5)
    in_maps = []
    for h in range(HEADS):
        Wq = W_qkv[:, h * DH : (h + 1) * DH] * scale
        Wk = W_qkv[:, DIM + h * DH : DIM + (h + 1) * DH]
        Wv = W_qkv[:, 2 * DIM + h * DH : 2 * DIM + (h + 1) * DH]
        wqk = chunked(np.concatenate([Wq, Wk], axis=1)).reshape(P, -1)
        wqkr = chunked(
            np.concatenate([fold_rot(Wq), fold_rot(Wk)], axis=1)
        ).reshape(P, -1)
        wv = chunked(Wv).reshape(P, -1)
        wout = np.zeros((P, DIM), dtype=bf16)
        wout[:DH] = W_out[h * DH : (h + 1) * DH, :].astype(bf16)
        consts = np.concatenate(
            [wqk, wqkr, wv, wout, cos2, sin2], axis=1
        ).astype(bf16)                                        # [P, CW + 2N]
        expb = np.ascontiguousarray(
            np.exp(pos_bias[h].T).reshape(KC, P, N).transpose(1, 0, 2)
        ).astype(bf16)                                        # [P, KC, N]
        in_maps.append(
            {
                "xT4": xT4,
                "consts": consts,
                "expb": expb,
            }
        )
    return in_maps


def kernel(x, pos_bias, W_qkv, W_out, _trace=False):
    from concourse.bass_utils import run_bass_kernel_spmd

    x = np.asarray(x, dtype=np.float32)
    pos_bias = np.asarray(pos_bias, dtype=np.float32)
    W_qkv = np.asarray(W_qkv, dtype=np.float32)
    W_out = np.asarray(W_out, dtype=np.float32)

    if "nc" not in _CACHE:
        _CACHE["nc"] = _build()
    nc = _CACHE["nc"]

    in_maps = _host_inputs(x, pos_bias, W_qkv, W_out)
    try:
        res = run_bass_kernel_spmd(
            nc, in_maps, core_ids=list(range(HEADS)), trace=_trace
        )
    except ModuleNotFoundError:
        res = run_bass_kernel_spmd(
            nc, in_maps, core_ids=list(range(HEADS)), trace=False
        )
    out = np.zeros((B, N, DIM), dtype=np.float32)
    for rmap in res.results:
        out += rmap["out"]
    if _trace:
        return out, res
    return out


if __name__ == "__main__":
    rng = np.random.default_rng(0)
    x = rng.standard_normal((B, N, DIM), dtype=np.float32)
    pb = rng.standard_normal((HEADS, N, N), dtype=np.float32)
    wq = rng.standard_normal((DIM, 3 * DIM), dtype=np.float32) * DIM**-0.5
    wo = rng.standard_normal((DIM, DIM), dtype=np.float32) * DIM**-0.5
    o = kernel(x, pb, wq, wo)
    print("kernel ran, out std:", o.std())
